# revision 28
# baseline (speedup 1.0000x reference)
"""Trainium2 Bass kernel for nn_BlockSelfAttentionModule.

Reference semantics (B=4, H=8, L=1024, I=16 instruments, F=64 frames, D=64):
  out[b*H+h, l, m] = q[l] . r_instrument[l%I, m%I, :, h]
                   + q[l] . a_h[(l//I - m//I) mod (F+1)]
  where a_h = concat(e_past[:, :, h], -111 pad row)   # (65, D)

Both bias terms factor through small per-row tables:
  Ui[l, c]  = q[l] . R_h[l%I, c]           (L x 16)
  Psh[l, f] = q[l] . a_h[(l//I - f) % 65]  (L x 64)
  out[l, f*16 + c] = Psh[l, f] + Ui[l, c]

Strategy (8 cores data-parallel over the 32 = B*H rows, 4 rows/core):
  The rank-2 outer-sum structure means the full (L, L) block never needs
  the classic materialize-and-store path (on-chip expansion + 25 us of
  SBUF->DRAM writes). Each core:
    1. computes Ui f-partitioned (16 PE matmuls, stationary = q columns of
       one instrument) and Psh l-partitioned (32 PE matmuls via the
       zero-padded zq / skewed a2 tables, K=128 covering 2 frames), with
       one DVE PSUM evacuation each,
    2. stores tiny staging tables to DRAM: uis2 (4L x 16 x 2, pair-doubled
       Ui) and pshs (4L x 64), then pshs4 (4L x 4 x 64, row-quadrupled via
       one broadcast DMA per row),
    3. expands them into the full c-major output (out[l, c*64+f]) with
       DRAM->DRAM DMAs whose DRAM-side dim0 is the whole l*c axis, so each
       costs ~500 ns regardless of bytes:
         - uis2 -> uis8 -> uis16 broadcast cascade, then 4 16-wide
           position writes per row out[l, c*64+16o+k] = uis16[l, c, k]
           (HWDGE; big descriptor counts are fine on SP/ACT),
         - 4 per-row gpsimd CCE-add accumulates out[...] += psh, reading
           pshs4's 256-element quadrupled runs so each stays at 4096 SWDGE
           descriptors (8192+ wedges the device ring).
  All sources keep a real contiguous fastest dim (stride-0 only in middle
  dims) and all lowered AP dim counts stay under the 16-bit ISA field
  limit - both are hard neuronxcc requirements.
Host casts the returned bf16 blocks to f32 and permutes the last axis
back from c-major to the reference n = f*16 + c order.
"""

import numpy as np
import ml_dtypes

import concourse.bass as bass
import concourse.bacc as bacc
import concourse.mybir as mybir

from concourse.tile import TileContext
from concourse.bass_utils import run_bass_kernel_spmd

F32 = mybir.dt.float32
BF16 = mybir.dt.bfloat16
NP_BF16 = ml_dtypes.bfloat16

N_CORES = 8
ROWS_PER_CORE = 4  # (b*H + h) rows per core
L = 1024
D = 64
I = 16
F = 64
PAD_VAL = -111.0

IT_COLS = 1024 + 256  # qT | rt

_PROGRAM = None

# Schedule config, tuned against CoreSim: input-load engine assignment and
# the emission order of per-row compute, staging stores, and expansions.
DEFAULT_CFG = {
    "loads": [
        ("a2", "pool"),
        ("it0", "sp"), ("zq0", "act"),
        ("it1", "sp"), ("zq1", "act"),
        ("it2", "sp"), ("zq2", "act"),
        ("it3", "sp"), ("zq3", "act"),
    ],
    "emit": (
        [("ui", 0), ("psh", 0), ("ui", 1), ("psh", 1),
         ("ui", 2), ("psh", 2), ("ui", 3), ("psh", 3)]
        + [("uis", 0, "act"), ("casc", 0, 512, "sp"),
           ("casc", 512, 512, "act"), ("casc2", 0, 1024, "sp"),
           ("pshs", 0, "pool"), ("dup", 0, "sp")]
        + [("oct", 0, 1024, o, ("sp", "act")[o % 2]) for o in range(4)]
        + [("acc", 0)]
        + [("uis", 1, "act"), ("casc", 1024, 512, "sp"),
           ("casc", 1536, 512, "act"), ("casc2", 1024, 1024, "act"),
           ("pshs", 1, "pool"), ("dup", 1, "sp")]
        + [("oct", 1024, 1024, o, ("sp", "act")[o % 2]) for o in range(4)]
        + [("acc", 1)]
        + [("uis", 2, "act"), ("casc", 2048, 512, "sp"),
           ("casc", 2560, 512, "act"), ("casc2", 2048, 1024, "sp"),
           ("pshs", 2, "pool"), ("dup", 2, "sp")]
        + [("oct", 2048, 1024, o, ("sp", "act")[o % 2]) for o in range(4)]
        + [("acc", 2)]
        + [("uis", 3, "act"), ("casc", 3072, 512, "sp"),
           ("casc", 3584, 512, "act"), ("casc2", 3072, 1024, "act"),
           ("pshs", 3, "pool"), ("dup", 3, "sp")]
        + [("oct", 3072, 1024, o, ("sp", "act")[o % 2]) for o in range(4)]
        + [("acc", 3)]
    ),
}


def build_program(loop_iters: int | None = None, cfg=None) -> bass.Bass:
    """loop_iters: device-side repeat loop for benchmarking only."""
    cfg = cfg or DEFAULT_CFG
    nc = bacc.Bacc("TRN2", debug=False, num_devices=N_CORES)
    qrtd = nc.declare_dram_parameter(
        "qrt", [ROWS_PER_CORE, D, IT_COLS], BF16, isOutput=False
    )
    zqd = nc.declare_dram_parameter(
        "zqd", [ROWS_PER_CORE, 128, L], BF16, isOutput=False
    )
    a2all = nc.declare_dram_parameter(
        "a2all", [128, ROWS_PER_CORE * 128], BF16, isOutput=False
    )
    uis2 = nc.declare_dram_parameter(
        "uis2", [ROWS_PER_CORE * L, I, 2], BF16, isOutput=True
    )
    uis8 = nc.declare_dram_parameter(
        "uis8", [ROWS_PER_CORE * L, I, 8], BF16, isOutput=True
    )
    uis16 = nc.declare_dram_parameter(
        "uis16", [ROWS_PER_CORE * L, I, 16], BF16, isOutput=True
    )
    pshs = nc.declare_dram_parameter(
        "pshs", [ROWS_PER_CORE * L, F], BF16, isOutput=True
    )
    pshs4 = nc.declare_dram_parameter(
        "pshs4", [ROWS_PER_CORE * L, 4, F], BF16, isOutput=True
    )
    out = nc.declare_dram_parameter(
        "out", [ROWS_PER_CORE, L, L], BF16, isOutput=True
    )

    with TileContext(nc) as tc:
        with (
            tc.tile_pool(name="const", bufs=1) as constp,
            tc.tile_pool(name="qrtp", bufs=ROWS_PER_CORE) as qrtp,
            tc.tile_pool(name="zqp", bufs=ROWS_PER_CORE) as zqp,
            tc.tile_pool(name="uip", bufs=2, space="PSUM") as uipp,
            tc.tile_pool(name="ps2", bufs=2, space="PSUM") as ps2p,
        ):
            a2t = constp.tile([128, ROWS_PER_CORE * 128], BF16)
            # pair-doubled l-major ui table:
            # uf2[f, j*512 + (i*16+c)*2 + k] = Ui_j[16f+i, c], k in {0,1}
            uf2 = constp.tile([F, ROWS_PER_CORE * 512], BF16)
            # l-partitioned psh for all rows: pshall[p, j*512 + lt*64 + f]
            pshall = constp.tile([128, ROWS_PER_CORE * 512], BF16)

            def body(_iv=None):
                eng = {"sp": nc.sync, "act": nc.scalar, "pool": nc.gpsimd}
                its = [qrtp.tile([D, IT_COLS], BF16, name=f"it{j}")
                       for j in range(ROWS_PER_CORE)]
                zqs = [zqp.tile([128, L], BF16, name=f"zq{j}")
                       for j in range(ROWS_PER_CORE)]
                for name, e in cfg["loads"]:
                    if name == "a2":
                        eng[e].dma_start(a2t[:], a2all[:])
                    elif name.startswith("it"):
                        j = int(name[2:])
                        eng[e].dma_start(its[j][:], qrtd[j])
                    else:
                        j = int(name[2:])
                        eng[e].dma_start(zqs[j][:], zqd[j])

                def emit_ui(j):
                    it = its[j]
                    # Ui, f-partitioned: stationary = q cols of instrument i,
                    # moving = R_i -> uip[f, 16i + c] = Ui[l = 16f + i, c]
                    qv = it[:, 0:1024].rearrange("d (f i) -> d i f", i=I)
                    uip = uipp.tile([F, 256], F32, name="uip")
                    for i in range(I):
                        nc.tensor.matmul(
                            uip[:, I * i : I * (i + 1)],
                            qv[:, i, :],
                            it[:, 1024 + I * i : 1024 + I * (i + 1)],
                        )
                    ov = (uf2[:, j * 512 : (j + 1) * 512]
                          .rearrange("f (ic k) -> f ic k", k=2))
                    iv = (uip[:].rearrange("f (ic o) -> f ic o", o=1)
                          .broadcast_to([F, 256, 2]))
                    nc.vector.tensor_copy(ov, iv)

                def emit_psh(j):
                    zq = zqs[j]
                    a2 = a2t[:, j * 128 : (j + 1) * 128]
                    # Psh: 32 matmuls into one [128, 512] PSUM tile.
                    # ps2[32g + r, 64lt + f] = Psh[l = lt*128 + 32g + r, f]
                    ps2 = ps2p.tile([128, 512], F32, name="ps2")
                    for lt in range(8):
                        for g in range(4):
                            fp = lt * 4 + g
                            nc.tensor.matmul(
                                ps2[32 * g : 32 * (g + 1),
                                    64 * lt : 64 * (lt + 1)],
                                zq[:, 32 * fp : 32 * (fp + 1)],
                                a2[:, 64 - 2 * fp : 128 - 2 * fp],
                                tile_position=(0, 32 * g),
                            )
                    nc.vector.tensor_copy(
                        pshall[:, j * 512 : (j + 1) * 512], ps2[:]
                    )

                outv = out[:].rearrange(
                    "j l (c f4 k) -> (j l) c f4 k", f4=4, k=16
                )

                def emit_uis2(j, e):
                    # uis2 store: src (f, (i c k)) -> uis2[l, c, k],
                    # l = j*L + 16f + i  (contiguous 512-elem runs both ways)
                    dst = (uis2[j * L : (j + 1) * L]
                           .rearrange("(f i) c k -> f (i c k)", i=I))
                    eng[e].dma_start(dst, uf2[:, j * 512 : (j + 1) * 512])

                def emit_casc(l0, nl, e):
                    # uis8[l, c, 4m+k] = uis2[l, c, k] over nl l-rows
                    # (nl <= 1023 keeps the merged dst dim under 65535)
                    sl = slice(l0, l0 + nl)
                    srcc = (uis2[sl].rearrange("l c k -> (l c) k")
                            .rearrange("lc (o k) -> lc o k", o=1)
                            .broadcast_to([nl * I, 4, 2]))
                    dstc = uis8[sl].rearrange("l c (m k) -> (l c) m k", k=2)
                    eng[e].dma_start(dstc, srcc)

                def emit_casc2(l0, nl, e):
                    # uis16[l, c, 8m+k] = uis8[l, c, k] over nl l-rows
                    # (nl <= 2047 keeps the merged dst dim under 65535)
                    sl = slice(l0, l0 + nl)
                    srcc = (uis8[sl].rearrange("l c k -> (l c) k")
                            .rearrange("lc (o k) -> lc o k", o=1)
                            .broadcast_to([nl * I, 2, 8]))
                    dstc = uis16[sl].rearrange("l c (m k) -> (l c) m k", k=8)
                    eng[e].dma_start(dstc, srcc)

                def emit_casc16(l0, nl, e):
                    # uis16[l, c, 2m+k] = uis2[l, c, k] in one x8 hop
                    # (nl <= 511 for the 16-bit field; 256 keeps HWDGE
                    # descriptors at the device-proven 32768)
                    sl = slice(l0, l0 + nl)
                    srcc = (uis2[sl].rearrange("l c k -> (l c) k")
                            .rearrange("lc (o k) -> lc o k", o=1)
                            .broadcast_to([nl * I, 8, 2]))
                    dstc = uis16[sl].rearrange("l c (m k) -> (l c) m k", k=2)
                    eng[e].dma_start(dstc, srcc)

                def emit_oct(l0, nl, o, e):
                    # ui write-expand, 16-wide position o over nl l-rows
                    # (nl <= 4095 keeps the dst dim0 under 65535):
                    # out[l, c*64 + 16o + k] = uis16[l, c, k]
                    dst = outv[l0 : l0 + nl][:, :, o, :]
                    srco = uis16[l0 : l0 + nl].rearrange("l c k -> (l c) k")
                    eng[e].dma_start(dst, srco)

                def emit_pshs(j, e):
                    dst = (pshs[j * L : (j + 1) * L, :]
                           .rearrange("(lt p) f -> p lt f", p=128))
                    eng[e].dma_start(
                        dst,
                        pshall[:, j * 512 : (j + 1) * 512]
                        .rearrange("p (lt f) -> p lt f", f=F),
                    )

                def emit_dup(j, e):
                    # pshs4[l, k, f] = pshs[l, f], k in 0..3: quadruples the
                    # contiguous run per l to 256 elems for the accum.
                    sl = slice(j * L, (j + 1) * L)
                    srcd = (pshs[sl].rearrange("(l o) f -> l o f", o=1)
                            .broadcast_to([L, 4, F]))
                    eng[e].dma_start(pshs4[sl], srcd)

                def emit_acc(j):
                    # psh accum over a full row, reading pshs4's 256-elem
                    # quadrupled runs -> 4096 SWDGE descriptors.
                    sl = slice(j * L, (j + 1) * L)
                    dst = (out[:]
                           .rearrange("j l (c4 kf) -> (j l) c4 kf", kf=256)
                           [j * L : (j + 1) * L])
                    srca = (pshs4[sl].rearrange("l k f -> l (k f)")
                            .rearrange("l (o kf) -> l o kf", o=1)
                            .broadcast_to([L, 4, 256]))
                    nc.gpsimd.dma_start(dst, srca,
                                        accum_op=mybir.AluOpType.add)

                for step in cfg["emit"]:
                    kind = step[0]
                    if kind == "ui":
                        emit_ui(step[1])
                    elif kind == "psh":
                        emit_psh(step[1])
                    elif kind == "uis":
                        emit_uis2(step[1], step[2])
                    elif kind == "casc":
                        emit_casc(step[1], step[2], step[3])
                    elif kind == "casc2":
                        emit_casc2(step[1], step[2], step[3])
                    elif kind == "casc16":
                        emit_casc16(step[1], step[2], step[3])
                    elif kind == "oct":
                        emit_oct(step[1], step[2], step[3], step[4])
                    elif kind == "pshs":
                        emit_pshs(step[1], step[2])
                    elif kind == "dup":
                        emit_dup(step[1], step[2])
                    elif kind == "acc":
                        emit_acc(step[1])

            if loop_iters is None:
                body()
            else:
                with tc.For_i(0, loop_iters, 1) as _iv:
                    body(_iv)
    return nc


def make_in_maps(q, r_instrument, e_past):
    """Host-side sharding + table prep. Returns per-core input dicts."""
    q = np.asarray(q, dtype=np.float32)
    r_instrument = np.asarray(r_instrument, dtype=np.float32)
    e_past = np.asarray(e_past, dtype=np.float32)

    qT = q.reshape(32, L, D).transpose(0, 2, 1)  # (32, D, L)

    # zq[r, s*64+d, l] = qT[r, d, l] where (l//16) % 2 == s, else 0
    par = (np.arange(L) // I) % 2
    zq = np.zeros((32, 128, L), np.float32)
    for s in (0, 1):
        cols = par == s
        zq[:, s * 64 : (s + 1) * 64, cols] = qT[:, :, cols]

    # rt[h, d, 16i+c] = R[i, c, d, h]
    rt = r_instrument.transpose(3, 2, 0, 1).reshape(8, D, I * I)  # (8, D, 256)

    # a2[h, s*64+d, t] = a_h[(64 - t + s) % 65, d], t in [0, 128)
    a = np.concatenate(
        [e_past, np.full((1, D, 8), PAD_VAL, dtype=np.float32)], axis=0
    )  # (65, D, H)
    idx2 = (64 - np.arange(128)[None, :] + np.arange(2)[:, None]) % 65
    a2 = a[idx2]  # (2, 128, D, 8)
    a2 = a2.transpose(3, 0, 2, 1).reshape(8, 128, 128)  # (h, s*64+d, t)

    in_maps = []
    for k in range(N_CORES):
        rows = [ROWS_PER_CORE * k + j for j in range(ROWS_PER_CORE)]
        hs = [r % 8 for r in rows]
        its = []
        for r, h in zip(rows, hs):
            its.append(
                np.concatenate([qT[r], rt[h]], axis=1)[None]
            )  # (1, 64, 1280)
        a2c = np.concatenate([a2[h] for h in hs], axis=1)  # (128, 512)
        in_maps.append(
            {
                "qrt": np.ascontiguousarray(
                    np.concatenate(its, axis=0).astype(NP_BF16)
                ),
                "a2all": np.ascontiguousarray(a2c.astype(NP_BF16)),
                "zqd": np.ascontiguousarray(zq[rows].astype(NP_BF16)),
            }
        )
    return in_maps


def _get_program() -> bass.Bass:
    global _PROGRAM
    if _PROGRAM is None:
        _PROGRAM = build_program()
        if not _PROGRAM.is_finalized():
            _PROGRAM.finalize()
    return _PROGRAM


def kernel(q, r_instrument, e_past, flipped_masks=None, **_unused):
    in_maps = make_in_maps(q, r_instrument, e_past)
    res = run_bass_kernel_spmd(_get_program(), in_maps, list(range(N_CORES))).results
    blocks = []
    for k in range(N_CORES):
        dev = np.asarray(res[k]["out"], dtype=np.float32)  # (4, L, L) c-major
        # device n' = c*64 + f  ->  reference n = f*16 + c
        blocks.append(
            dev.reshape(ROWS_PER_CORE, L, I, F)
            .transpose(0, 1, 3, 2)
            .reshape(ROWS_PER_CORE, L, L)
        )
    return np.ascontiguousarray(np.concatenate(blocks, axis=0))


# revision 31
# speedup vs baseline: 1.0551x; 1.0551x over previous
"""Trainium2 Bass kernel for nn_BlockSelfAttentionModule.

Reference semantics (B=4, H=8, L=1024, I=16 instruments, F=64 frames, D=64):
  out[b*H+h, l, m] = q[l] . r_instrument[l%I, m%I, :, h]
                   + q[l] . a_h[(l//I - m//I) mod (F+1)]
  where a_h = concat(e_past[:, :, h], -111 pad row)   # (65, D)

Both bias terms factor through small per-row tables:
  Ui[l, c]  = q[l] . R_h[l%I, c]           (L x 16)
  Psh[l, f] = q[l] . a_h[(l//I - f) % 65]  (L x 64)
  out[l, f*16 + c] = Psh[l, f] + Ui[l, c]

Strategy (8 cores data-parallel over the 32 = B*H rows, 4 rows/core):
  The rank-2 outer-sum structure means the full (L, L) block never needs
  the classic materialize-and-store path (on-chip expansion + 25 us of
  SBUF->DRAM writes). Each core:
    1. computes Ui f-partitioned (16 PE matmuls, stationary = q columns of
       one instrument) and Psh l-partitioned (32 PE matmuls via the
       zero-padded zq / skewed a2 tables, K=128 covering 2 frames), with
       one DVE PSUM evacuation each,
    2. stores tiny staging tables to DRAM: uis2 (4L x 16 x 2, pair-doubled
       Ui) and pshs (4L x 64), then pshs4 (4L x 4 x 64, row-quadrupled via
       one broadcast DMA per row),
    3. expands them into the full c-major output (out[l, c*64+f]) with
       DRAM->DRAM DMAs whose DRAM-side dim0 is the whole l*c axis, so each
       costs ~500 ns regardless of bytes:
         - uis2 -> uis8 -> uis16 broadcast cascade, then 4 16-wide
           position writes per row out[l, c*64+16o+k] = uis16[l, c, k]
           (HWDGE; big descriptor counts are fine on SP/ACT),
         - 4 per-row gpsimd CCE-add accumulates out[...] += psh, reading
           pshs4's 256-element quadrupled runs so each stays at 4096 SWDGE
           descriptors (8192+ wedges the device ring).
  All sources keep a real contiguous fastest dim (stride-0 only in middle
  dims) and all lowered AP dim counts stay under the 16-bit ISA field
  limit - both are hard neuronxcc requirements.
Host casts the returned bf16 blocks to f32 and permutes the last axis
back from c-major to the reference n = f*16 + c order.
"""

import numpy as np
import ml_dtypes

import concourse.bass as bass
import concourse.bacc as bacc
import concourse.mybir as mybir

from concourse.tile import TileContext
from concourse.bass_utils import run_bass_kernel_spmd

F32 = mybir.dt.float32
BF16 = mybir.dt.bfloat16
NP_BF16 = ml_dtypes.bfloat16

N_CORES = 8
ROWS_PER_CORE = 4  # (b*H + h) rows per core
L = 1024
D = 64
I = 16
F = 64
PAD_VAL = -111.0

IT_COLS = 1024 + 256  # qT | rt

_PROGRAM = None

# Schedule config, tuned against CoreSim: input-load engine assignment and
# the emission order of per-row compute, staging stores, and expansions.
DEFAULT_CFG = {
    "loads": [
        ("a2", "pool"),
        ("it0", "sp"), ("zq0", "act"),
        ("it1", "sp"), ("zq1", "act"),
        ("it2", "sp"), ("zq2", "act"),
        ("it3", "sp"), ("zq3", "act"),
    ],
    "emit": (
        [("ui", 0), ("psh", 0), ("ui", 1), ("psh", 1),
         ("ui", 2), ("psh", 2), ("ui", 3), ("psh", 3)]
        + [("uis", 0, "act"), ("casc", 0, 512, "sp"),
           ("casc", 512, 512, "act"), ("casc2", 0, 1024, "sp"),
           ("casc3", 0, 1024, "sp"),
           ("pshs", 0, "pool"), ("dup", 0, "sp"),
           ("oct32", 0, 1024, 0, "sp"), ("oct32", 0, 1024, 1, "act"),
           ("acc", 0)]
        + [("uis", 1, "act"), ("casc", 1024, 512, "sp"),
           ("casc", 1536, 512, "act"), ("casc2", 1024, 1024, "act"),
           ("casc3", 1024, 1024, "act"),
           ("pshs", 1, "pool"), ("dup", 1, "sp"),
           ("oct32", 1024, 1024, 0, "sp"), ("oct32", 1024, 1024, 1, "act"),
           ("acc", 1)]
        + [("uis", 2, "act"), ("casc", 2048, 512, "sp"),
           ("casc", 2560, 512, "act"), ("casc2", 2048, 1024, "sp"),
           ("casc3", 2048, 1024, "sp"),
           ("pshs", 2, "pool"), ("dup", 2, "sp"),
           ("oct32", 2048, 1024, 0, "sp"), ("oct32", 2048, 1024, 1, "act"),
           ("acc", 2)]
        + [("uis", 3, "act"), ("casc", 3072, 512, "sp"),
           ("casc", 3584, 512, "act"), ("casc2", 3072, 1024, "act"),
           ("casc3", 3072, 1024, "act"),
           ("pshs", 3, "pool"), ("dup", 3, "sp"),
           ("oct32", 3072, 1024, 0, "sp"), ("oct32", 3072, 1024, 1, "act"),
           ("acc", 3)]
    ),
}


def build_program(loop_iters: int | None = None, cfg=None) -> bass.Bass:
    """loop_iters: device-side repeat loop for benchmarking only."""
    cfg = cfg or DEFAULT_CFG
    nc = bacc.Bacc("TRN2", debug=False, num_devices=N_CORES)
    qrtd = nc.declare_dram_parameter(
        "qrt", [ROWS_PER_CORE, D, IT_COLS], BF16, isOutput=False
    )
    zqd = nc.declare_dram_parameter(
        "zqd", [ROWS_PER_CORE, 128, L], BF16, isOutput=False
    )
    a2all = nc.declare_dram_parameter(
        "a2all", [128, ROWS_PER_CORE * 128], BF16, isOutput=False
    )
    uis2 = nc.declare_dram_parameter(
        "uis2", [ROWS_PER_CORE * L, I, 2], BF16, isOutput=True
    )
    uis8 = nc.declare_dram_parameter(
        "uis8", [ROWS_PER_CORE * L, I, 8], BF16, isOutput=True
    )
    uis16 = nc.declare_dram_parameter(
        "uis16", [ROWS_PER_CORE * L, I, 16], BF16, isOutput=True
    )
    uis32 = nc.declare_dram_parameter(
        "uis32", [ROWS_PER_CORE * L, I, 32], BF16, isOutput=True
    )
    uis64 = nc.declare_dram_parameter(
        "uis64", [ROWS_PER_CORE * L, I, 64], BF16, isOutput=True
    )
    pshs = nc.declare_dram_parameter(
        "pshs", [ROWS_PER_CORE * L, F], BF16, isOutput=True
    )
    pshs4 = nc.declare_dram_parameter(
        "pshs4", [ROWS_PER_CORE * L, 4, F], BF16, isOutput=True
    )
    out = nc.declare_dram_parameter(
        "out", [ROWS_PER_CORE, L, L], BF16, isOutput=True
    )

    with TileContext(nc) as tc:
        with (
            tc.tile_pool(name="const", bufs=1) as constp,
            tc.tile_pool(name="qrtp", bufs=ROWS_PER_CORE) as qrtp,
            tc.tile_pool(name="zqp", bufs=ROWS_PER_CORE) as zqp,
            tc.tile_pool(name="uip", bufs=2, space="PSUM") as uipp,
            tc.tile_pool(name="ps2", bufs=2, space="PSUM") as ps2p,
        ):
            a2t = constp.tile([128, ROWS_PER_CORE * 128], BF16)
            # pair-doubled l-major ui table:
            # uf2[f, j*512 + (i*16+c)*2 + k] = Ui_j[16f+i, c], k in {0,1}
            uf2 = constp.tile([F, ROWS_PER_CORE * 512], BF16)
            # l-partitioned psh for all rows: pshall[p, j*512 + lt*64 + f]
            pshall = constp.tile([128, ROWS_PER_CORE * 512], BF16)

            def body(_iv=None):
                eng = {"sp": nc.sync, "act": nc.scalar, "pool": nc.gpsimd}
                its = [qrtp.tile([D, IT_COLS], BF16, name=f"it{j}")
                       for j in range(ROWS_PER_CORE)]
                zqs = [zqp.tile([128, L], BF16, name=f"zq{j}")
                       for j in range(ROWS_PER_CORE)]
                for name, e in cfg["loads"]:
                    if name == "a2":
                        eng[e].dma_start(a2t[:], a2all[:])
                    elif name.startswith("it"):
                        j = int(name[2:])
                        eng[e].dma_start(its[j][:], qrtd[j])
                    else:
                        j = int(name[2:])
                        eng[e].dma_start(zqs[j][:], zqd[j])

                def emit_ui(j):
                    it = its[j]
                    # Ui, f-partitioned: stationary = q cols of instrument i,
                    # moving = R_i -> uip[f, 16i + c] = Ui[l = 16f + i, c]
                    qv = it[:, 0:1024].rearrange("d (f i) -> d i f", i=I)
                    uip = uipp.tile([F, 256], F32, name="uip")
                    for i in range(I):
                        nc.tensor.matmul(
                            uip[:, I * i : I * (i + 1)],
                            qv[:, i, :],
                            it[:, 1024 + I * i : 1024 + I * (i + 1)],
                        )
                    ov = (uf2[:, j * 512 : (j + 1) * 512]
                          .rearrange("f (ic k) -> f ic k", k=2))
                    iv = (uip[:].rearrange("f (ic o) -> f ic o", o=1)
                          .broadcast_to([F, 256, 2]))
                    nc.vector.tensor_copy(ov, iv)

                def emit_psh(j):
                    zq = zqs[j]
                    a2 = a2t[:, j * 128 : (j + 1) * 128]
                    # Psh: 32 matmuls into one [128, 512] PSUM tile.
                    # ps2[32g + r, 64lt + f] = Psh[l = lt*128 + 32g + r, f]
                    ps2 = ps2p.tile([128, 512], F32, name="ps2")
                    for lt in range(8):
                        for g in range(4):
                            fp = lt * 4 + g
                            nc.tensor.matmul(
                                ps2[32 * g : 32 * (g + 1),
                                    64 * lt : 64 * (lt + 1)],
                                zq[:, 32 * fp : 32 * (fp + 1)],
                                a2[:, 64 - 2 * fp : 128 - 2 * fp],
                                tile_position=(0, 32 * g),
                            )
                    nc.vector.tensor_copy(
                        pshall[:, j * 512 : (j + 1) * 512], ps2[:]
                    )

                outv = out[:].rearrange(
                    "j l (c f4 k) -> (j l) c f4 k", f4=4, k=16
                )

                def emit_uis2(j, e):
                    # uis2 store: src (f, (i c k)) -> uis2[l, c, k],
                    # l = j*L + 16f + i  (contiguous 512-elem runs both ways)
                    dst = (uis2[j * L : (j + 1) * L]
                           .rearrange("(f i) c k -> f (i c k)", i=I))
                    eng[e].dma_start(dst, uf2[:, j * 512 : (j + 1) * 512])

                def emit_casc(l0, nl, e):
                    # uis8[l, c, 4m+k] = uis2[l, c, k] over nl l-rows
                    # (nl <= 1023 keeps the merged dst dim under 65535)
                    sl = slice(l0, l0 + nl)
                    srcc = (uis2[sl].rearrange("l c k -> (l c) k")
                            .rearrange("lc (o k) -> lc o k", o=1)
                            .broadcast_to([nl * I, 4, 2]))
                    dstc = uis8[sl].rearrange("l c (m k) -> (l c) m k", k=2)
                    eng[e].dma_start(dstc, srcc)

                def emit_casc2(l0, nl, e):
                    # uis16[l, c, 8m+k] = uis8[l, c, k] over nl l-rows
                    # (nl <= 2047 keeps the merged dst dim under 65535)
                    sl = slice(l0, l0 + nl)
                    srcc = (uis8[sl].rearrange("l c k -> (l c) k")
                            .rearrange("lc (o k) -> lc o k", o=1)
                            .broadcast_to([nl * I, 2, 8]))
                    dstc = uis16[sl].rearrange("l c (m k) -> (l c) m k", k=8)
                    eng[e].dma_start(dstc, srcc)

                def emit_casc16(l0, nl, e):
                    # uis16[l, c, 2m+k] = uis2[l, c, k] in one x8 hop
                    # (nl <= 511 for the 16-bit field; 256 keeps HWDGE
                    # descriptors at the device-proven 32768)
                    sl = slice(l0, l0 + nl)
                    srcc = (uis2[sl].rearrange("l c k -> (l c) k")
                            .rearrange("lc (o k) -> lc o k", o=1)
                            .broadcast_to([nl * I, 8, 2]))
                    dstc = uis16[sl].rearrange("l c (m k) -> (l c) m k", k=2)
                    eng[e].dma_start(dstc, srcc)

                def emit_casc3(l0, nl, e):
                    # uis32[l, c, 16m+k] = uis16[l, c, k] (nl <= 2047)
                    sl = slice(l0, l0 + nl)
                    srcc = (uis16[sl].rearrange("l c k -> (l c) k")
                            .rearrange("lc (o k) -> lc o k", o=1)
                            .broadcast_to([nl * I, 2, 16]))
                    dstc = uis32[sl].rearrange("l c (m k) -> (l c) m k", k=16)
                    eng[e].dma_start(dstc, srcc)

                def emit_oct32(l0, nl, o, e):
                    # ui write-expand, 32-wide position o (o in {0, 1})
                    dst = (out[:]
                           .rearrange("j l (c f2 k) -> (j l) c f2 k",
                                      f2=2, k=32)
                           [l0 : l0 + nl][:, :, o, :])
                    srco = uis32[l0 : l0 + nl].rearrange("l c k -> (l c) k")
                    eng[e].dma_start(dst, srco)

                def emit_casc4(l0, nl, e):
                    # uis64[l, c, 32m+k] = uis32[l, c, k] (nl <= 1023)
                    sl = slice(l0, l0 + nl)
                    srcc = (uis32[sl].rearrange("l c k -> (l c) k")
                            .rearrange("lc (o k) -> lc o k", o=1)
                            .broadcast_to([nl * I, 2, 32]))
                    dstc = uis64[sl].rearrange("l c (m k) -> (l c) m k", k=32)
                    eng[e].dma_start(dstc, srcc)

                def emit_w64(l0, nl, e):
                    # full ui write: out[l, c*64+k] = uis64[l, c, k]
                    dst = (out[:].rearrange("j l (c k) -> (j l) c k", k=64)
                           [l0 : l0 + nl])
                    srco = uis64[l0 : l0 + nl].rearrange("l c k -> (l c) k")
                    eng[e].dma_start(dst, srco)

                def emit_oct(l0, nl, o, e):
                    # ui write-expand, 16-wide position o over nl l-rows
                    # (nl <= 4095 keeps the dst dim0 under 65535):
                    # out[l, c*64 + 16o + k] = uis16[l, c, k]
                    dst = outv[l0 : l0 + nl][:, :, o, :]
                    srco = uis16[l0 : l0 + nl].rearrange("l c k -> (l c) k")
                    eng[e].dma_start(dst, srco)

                def emit_pshs(j, e):
                    dst = (pshs[j * L : (j + 1) * L, :]
                           .rearrange("(lt p) f -> p lt f", p=128))
                    eng[e].dma_start(
                        dst,
                        pshall[:, j * 512 : (j + 1) * 512]
                        .rearrange("p (lt f) -> p lt f", f=F),
                    )

                def emit_dup(j, e):
                    # pshs4[l, k, f] = pshs[l, f], k in 0..3: quadruples the
                    # contiguous run per l to 256 elems for the accum.
                    sl = slice(j * L, (j + 1) * L)
                    srcd = (pshs[sl].rearrange("(l o) f -> l o f", o=1)
                            .broadcast_to([L, 4, F]))
                    eng[e].dma_start(pshs4[sl], srcd)

                def emit_acc(j):
                    # psh accum over a full row, reading pshs4's 256-elem
                    # quadrupled runs -> 4096 SWDGE descriptors.
                    sl = slice(j * L, (j + 1) * L)
                    dst = (out[:]
                           .rearrange("j l (c4 kf) -> (j l) c4 kf", kf=256)
                           [j * L : (j + 1) * L])
                    srca = (pshs4[sl].rearrange("l k f -> l (k f)")
                            .rearrange("l (o kf) -> l o kf", o=1)
                            .broadcast_to([L, 4, 256]))
                    nc.gpsimd.dma_start(dst, srca,
                                        accum_op=mybir.AluOpType.add)

                for step in cfg["emit"]:
                    kind = step[0]
                    if kind == "ui":
                        emit_ui(step[1])
                    elif kind == "psh":
                        emit_psh(step[1])
                    elif kind == "uis":
                        emit_uis2(step[1], step[2])
                    elif kind == "casc":
                        emit_casc(step[1], step[2], step[3])
                    elif kind == "casc2":
                        emit_casc2(step[1], step[2], step[3])
                    elif kind == "casc16":
                        emit_casc16(step[1], step[2], step[3])
                    elif kind == "casc3":
                        emit_casc3(step[1], step[2], step[3])
                    elif kind == "oct32":
                        emit_oct32(step[1], step[2], step[3], step[4])
                    elif kind == "casc4":
                        emit_casc4(step[1], step[2], step[3])
                    elif kind == "w64":
                        emit_w64(step[1], step[2], step[3])
                    elif kind == "oct":
                        emit_oct(step[1], step[2], step[3], step[4])
                    elif kind == "pshs":
                        emit_pshs(step[1], step[2])
                    elif kind == "dup":
                        emit_dup(step[1], step[2])
                    elif kind == "acc":
                        emit_acc(step[1])

            if loop_iters is None:
                body()
            else:
                with tc.For_i(0, loop_iters, 1) as _iv:
                    body(_iv)
    return nc


def make_in_maps(q, r_instrument, e_past):
    """Host-side sharding + table prep. Returns per-core input dicts."""
    q = np.asarray(q, dtype=np.float32)
    r_instrument = np.asarray(r_instrument, dtype=np.float32)
    e_past = np.asarray(e_past, dtype=np.float32)

    qT = q.reshape(32, L, D).transpose(0, 2, 1)  # (32, D, L)

    # zq[r, s*64+d, l] = qT[r, d, l] where (l//16) % 2 == s, else 0
    par = (np.arange(L) // I) % 2
    zq = np.zeros((32, 128, L), np.float32)
    for s in (0, 1):
        cols = par == s
        zq[:, s * 64 : (s + 1) * 64, cols] = qT[:, :, cols]

    # rt[h, d, 16i+c] = R[i, c, d, h]
    rt = r_instrument.transpose(3, 2, 0, 1).reshape(8, D, I * I)  # (8, D, 256)

    # a2[h, s*64+d, t] = a_h[(64 - t + s) % 65, d], t in [0, 128)
    a = np.concatenate(
        [e_past, np.full((1, D, 8), PAD_VAL, dtype=np.float32)], axis=0
    )  # (65, D, H)
    idx2 = (64 - np.arange(128)[None, :] + np.arange(2)[:, None]) % 65
    a2 = a[idx2]  # (2, 128, D, 8)
    a2 = a2.transpose(3, 0, 2, 1).reshape(8, 128, 128)  # (h, s*64+d, t)

    in_maps = []
    for k in range(N_CORES):
        rows = [ROWS_PER_CORE * k + j for j in range(ROWS_PER_CORE)]
        hs = [r % 8 for r in rows]
        its = []
        for r, h in zip(rows, hs):
            its.append(
                np.concatenate([qT[r], rt[h]], axis=1)[None]
            )  # (1, 64, 1280)
        a2c = np.concatenate([a2[h] for h in hs], axis=1)  # (128, 512)
        in_maps.append(
            {
                "qrt": np.ascontiguousarray(
                    np.concatenate(its, axis=0).astype(NP_BF16)
                ),
                "a2all": np.ascontiguousarray(a2c.astype(NP_BF16)),
                "zqd": np.ascontiguousarray(zq[rows].astype(NP_BF16)),
            }
        )
    return in_maps


def _get_program() -> bass.Bass:
    global _PROGRAM
    if _PROGRAM is None:
        _PROGRAM = build_program()
        if not _PROGRAM.is_finalized():
            _PROGRAM.finalize()
    return _PROGRAM


def kernel(q, r_instrument, e_past, flipped_masks=None, **_unused):
    in_maps = make_in_maps(q, r_instrument, e_past)
    res = run_bass_kernel_spmd(_get_program(), in_maps, list(range(N_CORES))).results
    blocks = []
    for k in range(N_CORES):
        dev = np.asarray(res[k]["out"], dtype=np.float32)  # (4, L, L) c-major
        # device n' = c*64 + f  ->  reference n = f*16 + c
        blocks.append(
            dev.reshape(ROWS_PER_CORE, L, I, F)
            .transpose(0, 1, 3, 2)
            .reshape(ROWS_PER_CORE, L, L)
        )
    return np.ascontiguousarray(np.concatenate(blocks, axis=0))
